# revision 3
# baseline (speedup 1.0000x reference)
"""Trainium2 Bass kernel for nn_KSpaceLoss: exact type-2 NUFFT k-space loss.

v4: uint15 wrapped-phase chain, host-computed double-step deltas,
direct groups front-loaded, 4-chunk sin groups.

Math identical to v2/v3 (see kernel_v2 docstring). Pipeline:
 - groups: [seed(0,1), direct(36,37), direct(34,35)] emitted first (their
   inputs come straight from host DMAs), then 8 chained 4-chunk groups
   (chunks 2..33). PSUM accumulation: start on seed (first emitted),
   stop on the last chain group; transit+output DMA follow immediately.
 - chain: v[c] = (v[c-2] + dd2[(c-2)%3]) & 0x7FFF with host-computed
   rail-duplicated dd2 tiles; the four rail-pairs of a group update
   independently (two from the previous group, two intra) so the critical
   path per group is two VE hops while ACT runs one 4928-elem Sin.
"""

import math

import numpy as np
import ml_dtypes

import concourse.bacc as bacc
import concourse.tile as tile
from concourse import mybir
from concourse.bass_utils import run_bass_kernel_spmd

X, Y, Z = 96, 96, 1
C, S, T = 8, 1, 4
K = 8192
N = X * Y * Z
NCORES = 8
CST = C * S * T
W1, W2 = 0.1, 0.1

NCH = 37
NCHW = 38

F32 = mybir.dt.float32
F8 = mybir.dt.float8e4
U16 = mybir.dt.uint16
PI = math.pi
VSCALE = 32768.0


def build_kernel(kle):
    nc = bacc.Bacc("TRN2", target_bir_lowering=False, debug=False,
                   num_devices=NCORES)

    w1_d = nc.dram_tensor("w1", [128, NCHW, 64], F8, kind="ExternalInput").ap()
    w2_d = nc.dram_tensor("w2", [128, NCHW, 64], F8, kind="ExternalInput").ap()
    seed_d = nc.dram_tensor("seed", [128, 2, kle], U16, kind="ExternalInput").ap()
    dd2_d = nc.dram_tensor("dd2", [128, 3, kle], U16, kind="ExternalInput").ap()
    d34_d = nc.dram_tensor("d34", [128, 2, kle], U16, kind="ExternalInput").ap()
    d36_d = nc.dram_tensor("d36", [128, 1, kle], U16, kind="ExternalInput").ap()
    pso_d = nc.dram_tensor("pso", [64, kle], F32, kind="ExternalOutput").ap()

    Sin = mybir.ActivationFunctionType.Sin
    Alu = mybir.AluOpType
    DR = mybir.MatmulPerfMode.DoubleRow

    ksplits = []
    j = 0
    while j < kle:
        w = min(512, kle - j)
        ksplits.append(slice(j, j + w))
        j += w

    with tile.TileContext(nc) as tc:
        with (
            tc.tile_pool(name="const", bufs=1) as cpool,
            tc.tile_pool(name="qq", bufs=4) as qpool,
            tc.tile_pool(name="acc", bufs=1, space="PSUM") as accp,
            tc.tile_pool(name="ework", bufs=3) as ewp,
            tc.tile_pool(name="resid", bufs=1) as rsp,
        ):
            bias_npi = cpool.tile([128, 1], F32, tag="bnpi")
            nc.vector.memset(bias_npi[:], -PI)
            dd2 = cpool.tile([128, 3, kle], U16, tag="dd2")
            w1 = cpool.tile([128, NCHW, 64], F8, tag="w1")
            w2 = cpool.tile([128, NCHW, 64], F8, tag="w2")

            ps = accp.tile([64, kle], F32, tag="ps")

            # (first_chunk, nchunks, kind, src_group_index)
            GROUPS = ([(0, 2, "seed", None), (36, 1, "d36", None),
                       (34, 2, "d34", None)]
                      + [(2 + 4 * i, 4, "chain", 0 if i == 0 else 2 + i)
                         for i in range(8)])
            NG = len(GROUPS)

            qtiles = {}
            etiles = {}

            def rails(t, j, w):
                return t[:, j::w, :]

            def emit_q(gi):
                c0, nch, kind, src = GROUPS[gi]
                qq = qpool.tile([128, 2 * nch, kle], U16, tag=f"qq{kind}")
                qtiles[gi] = qq
                def derive_er(nch2, dram):
                    # s-rails -> planes [nch2:2*nch2]; er = (s + 8192) & M
                    nc.sync.dma_start(qq[:, nch2:2 * nch2, :], dram)
                    t = qpool.tile([128, 2, kle], U16, tag="qder",
                                   bufs=3)
                    te = t[:, 0:nch2, :]
                    nc.vector.tensor_scalar(te, qq[:, nch2:2 * nch2, :],
                                            8192, None, op0=Alu.add)
                    nc.vector.tensor_scalar(qq[:, 0:nch2, :], te, 0x7FFF,
                                            None, op0=Alu.bitwise_and)
                if kind == "seed":
                    derive_er(2, seed_d[:])
                    return
                if kind == "d36":
                    derive_er(1, d36_d[:])
                    return
                if kind == "d34":
                    derive_er(2, d34_d[:])
                    return
                pq = qtiles[src]
                pn = GROUPS[src][1]
                srcs = [rails(pq, pn - 2, pn), rails(pq, pn - 1, pn)]
                tmps = []
                for j in range(2):
                    t = qpool.tile([128, 2, kle], U16, tag=f"qt{j}")
                    nc.vector.tensor_tensor(
                        t[:], srcs[j],
                        dd2[:, (c0 - 2 + j) % 3, :].unsqueeze(1)
                        .broadcast_to([128, 2, kle]),
                        op=Alu.add)
                    tmps.append(t)
                for j in range(2):
                    nc.vector.tensor_scalar(rails(qq, j, 4), tmps[j][:],
                                            0x7FFF, None,
                                            op0=Alu.bitwise_and)
                tmps2 = []
                for j in range(2):
                    t = qpool.tile([128, 2, kle], U16, tag=f"qu{j}")
                    nc.vector.tensor_tensor(
                        t[:], rails(qq, j, 4),
                        dd2[:, (c0 + j) % 3, :].unsqueeze(1)
                        .broadcast_to([128, 2, kle]),
                        op=Alu.add)
                    tmps2.append(t)
                for j in range(2):
                    nc.vector.tensor_scalar(rails(qq, 2 + j, 4),
                                            tmps2[j][:], 0x7FFF, None,
                                            op0=Alu.bitwise_and)

            def emit_sins(gi):
                c0, nch, kind, src = GROUPS[gi]
                qq = qtiles[gi]
                ee = ewp.tile([128, 2 * nch, kle], F8, tag=f"ee{nch}")
                etiles[gi] = ee
                nc.scalar.activation(ee[:], qq[:], Sin, bias=bias_npi[:],
                                     scale=float(2 * PI / VSCALE))

            def emit_back(gi):
                c0, nch, kind, src = GROUPS[gi]
                ee = etiles.pop(gi)
                if nch == 1:
                    wsl = slice(c0, c0 + 1)
                    for sl in ksplits:
                        nc.tensor.matmul(ps[:, sl], w1[:, wsl, :],
                                         ee[:, 0:1, sl],
                                         start=False, stop=False)
                    for sl in ksplits:
                        nc.tensor.matmul(ps[:, sl], w2[:, wsl, :],
                                         ee[:, 1:2, sl],
                                         start=False, stop=False)
                    return
                for h in range(nch // 2):
                    cc = c0 + 2 * h
                    first = gi == 0 and h == 0
                    last = gi == NG - 1 and h == nch // 2 - 1
                    wsl = slice(cc, cc + 2)
                    for sl in ksplits:
                        nc.tensor.matmul(ps[:, sl], w1[:, wsl, :],
                                         ee[:, 2 * h:2 * h + 2, sl],
                                         start=first, stop=False,
                                         perf_mode=DR)
                    for sl in ksplits:
                        nc.tensor.matmul(ps[:, sl], w2[:, wsl, :],
                                         ee[:, nch + 2 * h:nch + 2 * h + 2, sl],
                                         start=False, stop=last,
                                         perf_mode=DR)

            # tiny ACT-critical tiles on sync; dd2/weights on gpsimd
            emit_q(1)
            emit_q(0)
            emit_q(2)
            nc.gpsimd.dma_start(dd2[:], dd2_d[:])
            nc.gpsimd.dma_start(w1[:], w1_d[:])
            nc.gpsimd.dma_start(w2[:], w2_d[:])

            PF = 1
            for t in range(1, NG + 1 + PF):
                if 3 <= t < NG:
                    emit_q(t)
                if t <= NG:
                    emit_sins(t - 1)
                if t >= 1 + PF:
                    emit_back(t - 1 - PF)
            qtiles.clear()

            psS = rsp.tile([64, kle], F32, tag="psS")
            nc.vector.tensor_scalar(psS[:], ps[:], 0.0, None, op0=Alu.add)
            nc.sync.dma_start(pso_d[:], psS[:])

    nc.compile()
    return nc


_NC_CACHE = {}


def _get_nc(kle):
    if kle not in _NC_CACHE:
        _NC_CACHE[kle] = build_kernel(kle)
    return _NC_CACHE[kle]


def _rep_layout():
    gy_main = np.repeat(np.arange(1, 48), 96)
    gx_main = np.tile(np.arange(-48, 48), 47)
    gy_bnd = np.concatenate([np.full(96, -48), np.arange(-47, 1),
                             np.zeros(48, np.int64)])
    gx_bnd = np.concatenate([np.arange(-48, 48), np.full(48, -48),
                             np.arange(0, 48)])
    gxs = np.concatenate([gx_main, gx_bnd])
    gys = np.concatenate([gy_main, gy_bnd])
    npts = gxs.size
    pad = NCH * 128 - npts
    gxs = np.concatenate([gxs, np.zeros(pad, np.int64)])
    gys = np.concatenate([gys, np.zeros(pad, np.int64)])
    return gxs, gys, npts


def _host_prep(images_reconstructed, kspace_trajectory, kspace_data,
               kspace_mask, sensitivity_maps):
    img = np.asarray(images_reconstructed)
    traj = np.asarray(kspace_trajectory).astype(np.float64)
    kdata = np.asarray(kspace_data)
    mask = np.asarray(kspace_mask).astype(np.float32)
    smaps = np.asarray(sensitivity_maps)
    f8 = ml_dtypes.float8_e4m3

    gxs, gys, npts = _rep_layout()

    x = 0.5 * img[None, ...] * smaps[..., None, None]
    xw = x.reshape(C, N, T).transpose(1, 0, 2).reshape(N, CST)
    rn = (gxs + 48) * 96 + (gys + 48)
    has_m = (gxs >= -47) & (gys >= -47) & ~((gxs == 0) & (gys == 0))
    has_m[npts:] = False
    mn = np.where(has_m, (np.where(has_m, -gxs, 0) + 48) * 96
                  + (np.where(has_m, -gys, 0) + 48), 0)
    xr = xw.real.astype(np.float32)
    xi = xw.imag.astype(np.float32)
    xr_r = xr[rn]
    xi_r = xi[rn]
    xr_r[npts:] = 0.0
    xi_r[npts:] = 0.0
    on = has_m[:, None]
    xr_m = np.where(on, xr[mn], 0.0)
    xi_m = np.where(on, xi[mn], 0.0)
    w1 = np.concatenate([xr_r + xr_m, xi_r + xi_m], 1)
    w2 = -np.concatenate([-(xi_r - xi_m), xr_r - xr_m], 1)
    zpad = np.zeros((128, 64), np.float32)
    w1 = np.ascontiguousarray(np.vstack([w1, zpad]).astype(f8)
                              .reshape(NCHW, 128, 64).transpose(1, 0, 2))
    w2 = np.ascontiguousarray(np.vstack([w2, zpad]).astype(f8)
                              .reshape(NCHW, 128, 64).transpose(1, 0, 2))

    mk = mask.reshape(K)
    act = np.nonzero(mk)[0]
    ke = act.size
    kle = -(-ke // NCORES)
    kle = max(8 * (-(-kle // 8)), 128)
    tx = np.zeros(NCORES * kle)
    ty = np.zeros(NCORES * kle)
    tx[:ke] = traj[0][act]
    ty[:ke] = traj[1][act]
    kd = kdata.reshape(C, K, T).transpose(1, 0, 2).reshape(K, CST)[act]
    mkact = mk[act].astype(np.float64)

    def vq(ph, off):
        v = np.floor((ph + off) * VSCALE + 0.5)
        return np.mod(v, VSCALE).astype(np.uint16)

    gxp = gxs[:128 * 36]
    gyp = gys[:128 * 36]
    dpats = []
    for m3 in range(3):
        nn = np.arange(128 * m3, 128 * m3 + 128)
        dpats.append((gxp[nn + 128] - gxp[nn], gyp[nn + 128] - gyp[nn]))

    in_maps = []
    for i in range(NCORES):
        ksl = slice(i * kle, (i + 1) * kle)
        txc, tyc = tx[ksl], ty[ksl]

        def phase(psl):
            return (gxs[psl, None] * txc[None, :]
                    + gys[psl, None] * tyc[None, :])

        def railpair(c):
            ph = phase(slice(128 * c, 128 * (c + 1)))
            return vq(ph, 0.75), vq(ph, 0.5)

        _, s0 = railpair(0)
        _, s1 = railpair(1)
        seed = np.stack([s0, s1], 1)
        _, s34 = railpair(34)
        _, s35 = railpair(35)
        d34 = np.stack([s34, s35], 1)
        _, s36 = railpair(36)
        d36 = s36[:, None, :]
        # dd2[m] = (delta[m] + delta[(m+1)%3]) mod 2^15 (single rail)
        dd2 = np.empty((128, 3, kle), np.uint16)
        dv = []
        for dgx, dgy in dpats:
            dv.append(vq(dgx[:, None] * txc[None, :]
                         + dgy[:, None] * tyc[None, :], 0.0).astype(np.int64))
        for m in range(3):
            dd2[:, m, :] = (dv[m] + dv[(m + 1) % 3]) & 0x7FFF
        in_maps.append({
            "w1": w1, "w2": w2,
            "seed": np.ascontiguousarray(seed),
            "dd2": dd2,
            "d34": np.ascontiguousarray(d34),
            "d36": np.ascontiguousarray(d36),
        })
    return in_maps, kd, mkact, ke, kle


def kernel(images_reconstructed, kspace_trajectory, kspace_data,
           kspace_mask, sensitivity_maps, _trace=False):
    in_maps, kd, mkact, ke, kle = _host_prep(
        images_reconstructed, kspace_trajectory, kspace_data,
        kspace_mask, sensitivity_maps)
    nc = _get_nc(kle)
    if _trace:
        import tempfile
        kw = {"tmpdir": tempfile.mkdtemp(prefix="/tmp/bass_trace_")}
    else:
        kw = {}
    res = run_bass_kernel_spmd(nc, in_maps, core_ids=list(range(NCORES)),
                               trace=_trace, **kw)
    pso = np.concatenate([res.results[i]["pso"] for i in range(NCORES)],
                         axis=1)[:, :ke]
    ksp = (pso[:CST] + 1j * pso[CST:]).T.astype(np.complex128)
    d = ksp * mkact[:, None] - kd * mkact[:, None]
    ad = np.abs(d)
    l1, l2 = ad.sum(), (ad * ad).sum()
    a = np.abs(kd * mkact[:, None])
    a1, a2 = a.sum(), (a * a).sum()
    loss = np.asarray(W1 * (l1 / a1) + W2 * math.sqrt(l2) / math.sqrt(a2),
                      dtype=np.float32)
    if _trace:
        return loss, res
    return loss
